# revision 1
# baseline (speedup 1.0000x reference)
"""Mixture-of-Experts (top-1 routing) Trainium2 kernel.

Strategy (expert-parallel, per sharding hint):
 - Router (softmax / argmax / top-prob) evaluated on host — 8192x8, i.e.
   0.002% of the FLOPs; its cost is dispatch bookkeeping.
 - Tokens are dispatched to the core owning their expert: core e receives
   the tokens routed to expert e (transposed, padded to capacity C), plus
   W[e], b[e] and the per-token gate probability.
 - Each core runs a dense [C,1024] @ [1024,1024] GEMM on the TensorEngine
   in float32r (full-rate fp32, ~1.5e-4 max rel err), adds the bias,
   scales rows by the gate probability, and writes the compact result.
 - Host scatters the compact per-expert outputs back to token order
   (the "second all-to-all" / unshard step).
"""

import numpy as np

T, H, E = 8192, 1024, 8
N_CORES = 8
P = 128
KT = H // P          # 8 contraction tiles
NFREE = 512          # matmul moving free dim (one PSUM bank of fp32)
NT = H // NFREE      # 2 output column tiles

_BUILD_CACHE = {}


def _build(C):
    """Build the SPMD Bass module for per-core token capacity C (multiple of 128)."""
    import concourse.mybir as mybir
    import concourse.tile as tile
    from concourse import bacc

    MT = C // P
    DT = mybir.dt.float32r   # fp32 bits, full-rate matmul
    F32 = mybir.dt.float32

    nc = bacc.Bacc("TRN2", target_bir_lowering=False, debug=False,
                   num_devices=N_CORES)

    xt_d = nc.dram_tensor("xt", [KT, P, C], DT, kind="ExternalInput").ap()
    w_d = nc.dram_tensor("w", [KT, P, H], DT, kind="ExternalInput").ap()
    bias_d = nc.dram_tensor("bias", [P, H], F32, kind="ExternalInput").ap()
    scale_d = nc.dram_tensor("scale", [MT, P], F32, kind="ExternalInput").ap()
    out_d = nc.dram_tensor("out", [MT, P, H], F32, kind="ExternalOutput").ap()

    with tile.TileContext(nc) as tc:
        with (
            tc.tile_pool(name="ins", bufs=1) as ins,
            tc.tile_pool(name="psum", bufs=1, space="PSUM") as psum_pool,
            tc.tile_pool(name="outp", bufs=4) as outp,
        ):
            xt_sb = [ins.tile([P, C], DT, name=f"xt{k}") for k in range(KT)]
            w_sb = [ins.tile([P, H], DT, name=f"w{k}") for k in range(KT)]
            bias_sb = ins.tile([P, H], F32, name="bias")
            scale_sb = ins.tile([P, MT], F32, name="scale")

            # interleave input DMAs so k-tile pairs arrive together
            for k in range(KT):
                nc.sync.dma_start(xt_sb[k][:], xt_d[k])
                nc.sync.dma_start(w_sb[k][:], w_d[k])
            nc.sync.dma_start(bias_sb[:], bias_d[:])
            nc.sync.dma_start(scale_sb[:], scale_d.rearrange("m p -> p m"))

            m_chunks = [list(range(s, min(s + 4, MT))) for s in range(0, MT, 4)]
            for chunk in m_chunks:
                ps = {}
                for ci, m in enumerate(chunk):
                    for n in range(NT):
                        ps[m, n] = psum_pool.tile([P, NFREE], F32,
                                                  name=f"ps{ci}_{n}")
                for k in range(KT):
                    for m in chunk:
                        for n in range(NT):
                            nc.tensor.matmul(
                                ps[m, n][:],
                                xt_sb[k][:, m * P:(m + 1) * P],
                                w_sb[k][:, n * NFREE:(n + 1) * NFREE],
                                start=(k == 0), stop=(k == KT - 1),
                            )
                for m in chunk:
                    for n in range(NT):
                        nsl = slice(n * NFREE, (n + 1) * NFREE)
                        t = outp.tile([P, NFREE], F32, name="osb")
                        nc.vector.tensor_add(t[:], ps[m, n][:], bias_sb[:, nsl])
                        nc.vector.tensor_scalar_mul(t[:], t[:],
                                                    scale_sb[:, m:m + 1])
                        nc.sync.dma_start(out_d[m][:, nsl], t[:])

    nc.compile()
    return nc


def kernel(input, gate, W, b):
    from concourse import bass_utils

    input = np.ascontiguousarray(input, dtype=np.float32)
    gate = np.ascontiguousarray(gate, dtype=np.float32)
    W = np.ascontiguousarray(W, dtype=np.float32)
    b = np.ascontiguousarray(b, dtype=np.float32)

    # ---- router (host): top-1 expert + its softmax probability ----
    g = gate.astype(np.float64)
    gm = g.max(axis=1, keepdims=True)
    top_p = (1.0 / np.exp(g - gm).sum(axis=1)).astype(np.float32)
    e_t = np.argmax(gate, axis=1)

    counts = np.bincount(e_t, minlength=E)
    order = np.argsort(e_t, kind="stable")
    starts = np.zeros(E + 1, dtype=np.int64)
    np.cumsum(counts, out=starts[1:])

    C = max(P, int(-(-counts.max() // P)) * P)
    MT = C // P

    if C not in _BUILD_CACHE:
        _BUILD_CACHE[C] = _build(C)
    nc = _BUILD_CACHE[C]

    in_maps = []
    ids_per_e = []
    for e in range(E):
        ids = order[starts[e]:starts[e + 1]]
        ids_per_e.append(ids)
        n_e = len(ids)

        xt = np.zeros((KT, P, C), dtype=np.float32)
        xt.reshape(H, C)[:, :n_e] = input[ids].T

        scale = np.zeros((MT, P), dtype=np.float32)
        scale.reshape(C)[:n_e] = top_p[ids]

        in_maps.append({
            "xt": xt,
            "w": W[e].reshape(KT, P, H),
            "bias": np.ascontiguousarray(np.broadcast_to(b[e], (P, H))),
            "scale": scale,
        })

    res = bass_utils.run_bass_kernel_spmd(nc, in_maps,
                                          core_ids=list(range(N_CORES)))

    out = np.empty((T, H), dtype=np.float32)
    for e in range(E):
        ids = ids_per_e[e]
        out[ids] = res.results[e]["out"].reshape(C, H)[:len(ids)]
    return out
